# revision 6
# baseline (speedup 1.0000x reference)
"""AttentionAggregator kernel for 8 trn2 NeuronCores — v3.

Math (wa-prescaling, as v1): wa = W @ a; HBM holds ns = neigh*wa (bf16) in
d-major per-node layout [node, (d k)]; ss = self*wa; W' = W/wa rows.

d-major layout benefits vs v1:
  - logits: 8 identity matmuls, k-inner streaming -> PSUM address-repeat over
    d with no same-cycle address collision and no pair-parity merge; output
    is directly [P, K].
  - E-scale: the e2 broadcast AP has k (stride 1, bf16) innermost -> DVE 2x
    mode guaranteed; ONE tensor_tensor over [P, D*K].
  - combine: d-inner streaming accumulates k into [P, D] with no parity.
Engine moves vs v1: self-logit on PE (2-addr parity row-sum), E-sum via exp's
accum_out (ACT), e2 = r*E on ACT (verified Copy-with-scale-AP), agg drain on
ACT, output in bf16.
"""

import sys

sys.path.insert(0, "/opt/trn_rl_repo")

import numpy as np
import ml_dtypes

import concourse.bass as bass
import concourse.bacc as bacc
import concourse.mybir as mybir
import concourse.tile as tile
from concourse.bass_utils import run_bass_kernel_spmd

N_CORES = 8
D = 128
K = 32
P = 128
TILES = 49
NODES_PC = TILES * P             # 6272
ROWS_PC = NODES_PC * K           # 200704
N_FULL = 50000

F32 = mybir.dt.float32
BF16 = mybir.dt.bfloat16
BF = ml_dtypes.bfloat16

_cache = {}


def _build(reps=1, skip=()):
    nc = bacc.Bacc("TRN2", target_bir_lowering=False, debug=False)

    neigh_t = nc.dram_tensor("neigh_bf", [NODES_PC, D * K], BF16, kind="ExternalInput")
    ss_t = nc.dram_tensor("ss_bf", [NODES_PC, D], BF16, kind="ExternalInput")
    w_t = nc.dram_tensor("w2_bf", [D, D], BF16, kind="ExternalInput")
    ident_t = nc.dram_tensor("ident_bf", [P, P], BF16, kind="ExternalInput")
    ones_t = nc.dram_tensor("ones_bf", [1, P], BF16, kind="ExternalInput")
    bias_t = nc.dram_tensor("bias_bf", [1, D], BF16, kind="ExternalInput")
    out_t = nc.dram_tensor("out", [NODES_PC, D], BF16, kind="ExternalOutput")

    with tile.TileContext(nc) as tc:
        with (
            tc.tile_pool(name="const", bufs=1) as cpool,
            tc.tile_pool(name="big", bufs=1) as bigpool,
            tc.tile_pool(name="nb", bufs=4) as nbpool,
            tc.tile_pool(name="work", bufs=3) as wpool,
            tc.tile_pool(name="small", bufs=6) as smpool,
            tc.tile_pool(name="ps_log", bufs=2, space="PSUM") as ps_log,
            tc.tile_pool(name="ps_sl", bufs=2, space="PSUM") as ps_sl,
            tc.tile_pool(name="ps_agg", bufs=2, space="PSUM") as ps_agg,
            tc.tile_pool(name="ps_fin", bufs=1, space="PSUM") as ps_fin,
        ):
            ident = cpool.tile([P, P], BF16)
            w_sb = cpool.tile([D, D], BF16)
            ones_sb = cpool.tile([1, P], BF16)
            bias_sb = cpool.tile([1, D], BF16)
            nc.sync.dma_start(ident[:], ident_t[:])
            nc.sync.dma_start(w_sb[:], w_t[:])
            nc.sync.dma_start(ones_sb[:], ones_t[:])
            nc.sync.dma_start(bias_sb[:], bias_t[:])

            ss_sb = bigpool.tile([P, TILES * D], BF16)
            out_big = bigpool.tile([P, TILES * D], BF16)
            t0 = 0
            while t0 < TILES:
                q = min(4, TILES - t0)
                nc.sync.dma_start(
                    ss_sb[:, t0 * D : (t0 + q) * D].rearrange(
                        "p (q d) -> p q d", q=q
                    ),
                    ss_t[t0 * P : (t0 + q) * P, :].rearrange(
                        "(q p) d -> p q d", p=P
                    ),
                )
                t0 += q

            for rep in range(reps):
              for t in range(TILES):
                nb = nbpool.tile([P, D * K], BF16, tag="nb")
                if "dma" in skip and t > 0 and rep > 0:
                    pass
                else:
                    nc.sync.dma_start(nb[:], neigh_t[t * P : (t + 1) * P, :])
                sf = ss_sb[:, t * D : (t + 1) * D]

                # ---- logits: 8 identity matmuls, k-inner streaming ----
                log_ps = ps_log.tile([P, K], F32, tag="log_ps")
                for g in range(8 if "logits" not in skip else 1):
                    out_ap = log_ps[:].unsqueeze(1).broadcast_to((P, 16, K))
                    nc.tensor.matmul(
                        out_ap,
                        ident[:],
                        nb[:, g * 16 * K : (g + 1) * 16 * K],
                        start=(g == 0),
                        stop=(g == 7),
                    )

                # ---- self logit: PE row-sum with 2-address parity ----
                sl_ps = ps_sl.tile([P, 2], F32, tag="sl_ps")
                nc.tensor.matmul(
                    sl_ps[:].unsqueeze(1).broadcast_to((P, D // 2, 2)),
                    ident[:],
                    sf,
                )

                # ---- softmax front ----
                a_sb = smpool.tile([P, K], F32, tag="a_sb")
                nc.vector.tensor_scalar(
                    a_sb[:], log_ps[:], sl_ps[:, 0:1], sl_ps[:, 1:2],
                    mybir.AluOpType.add, mybir.AluOpType.add,
                )
                l_sb = smpool.tile([P, K], F32, tag="l_sb")
                nc.vector.scalar_tensor_tensor(
                    l_sb[:], a_sb[:], 0.2, a_sb[:],
                    mybir.AluOpType.mult, mybir.AluOpType.max,
                )
                e_sb = smpool.tile([P, K], BF16, tag="e_sb")
                s_sb = smpool.tile([P, 1], F32, tag="s_sb")
                nc.scalar.activation(
                    e_sb[:], l_sb[:], mybir.ActivationFunctionType.Exp,
                    accum_out=s_sb[:],
                )
                r_sb = smpool.tile([P, 1], F32, tag="r_sb")
                nc.vector.reciprocal(r_sb[:], s_sb[:])
                e2_sb = smpool.tile([P, K], BF16, tag="e2_sb")
                nc.scalar.activation(
                    e2_sb[:], e_sb[:], mybir.ActivationFunctionType.Copy,
                    scale=r_sb[:, 0:1],
                )

                # ---- sc = nb * e2: ONE DVE op at 2x ----
                sc = wpool.tile([P, D * K], BF16, tag="sc")
                ebc = e2_sb[:].unsqueeze(1).broadcast_to((P, D, K))
                nc.vector.tensor_tensor(
                    sc[:], nb[:], ebc, mybir.AluOpType.mult
                )

                # ---- combine: 8 matmuls, d-inner streaming, no parity ----
                agg_ps = ps_agg.tile([P, D], F32, tag="agg_ps")
                for g in range(8 if "combine" not in skip else 1):
                    rhs = sc[:, g * 16 * K : (g + 1) * 16 * K].rearrange(
                        "p (d k) -> p k d", k=K
                    )
                    out_ap = (
                        agg_ps[:, g * 16 : (g + 1) * 16]
                        .unsqueeze(1)
                        .broadcast_to((P, K, 16))
                    )
                    nc.tensor.matmul(out_ap, ident[:], rhs)

                # ---- sn = ss + agg ; transpose; @W' + bias; relu ----
                rag_sb = smpool.tile([P, D], BF16, tag="rag_sb")
                nc.scalar.activation(
                    rag_sb[:], agg_ps[:], mybir.ActivationFunctionType.Copy
                )
                sn_sb = smpool.tile([P, D], BF16, tag="sn_sb")
                nc.vector.tensor_tensor(
                    sn_sb[:], rag_sb[:], sf, mybir.AluOpType.add
                )
                snt_ps = ps_fin.tile([P, D], F32, tag="snt_ps")
                nc.tensor.matmul(snt_ps[:], sn_sb[:], ident[:])
                snt_sb = smpool.tile([P, D], BF16, tag="snt_sb")
                nc.scalar.copy(snt_sb[:], snt_ps[:])

                o_ps = ps_fin.tile([P, D], F32, tag="o_ps")
                nc.tensor.matmul(o_ps[:], ones_sb[:], bias_sb[:], start=True, stop=False)
                nc.tensor.matmul(o_ps[:], snt_sb[:], w_sb[:], start=False, stop=True)
                nc.scalar.activation(
                    out_big[:, t * D : (t + 1) * D], o_ps[:],
                    mybir.ActivationFunctionType.Relu,
                )
                if t % 4 == 3 or t == TILES - 1:
                    t0g = (t // 4) * 4
                    qg = t - t0g + 1
                    nc.sync.dma_start(
                        out_t[t0g * P : (t0g + qg) * P, :].rearrange(
                            "(q p) d -> p q d", p=P
                        ),
                        out_big[:, t0g * D : (t + 1) * D].rearrange(
                            "p (q d) -> p q d", q=qg
                        ),
                    )

    nc.compile()
    return nc


def _prep(self_vecs, neigh_vecs, feat_weights, attn_weights, bias):
    n = self_vecs.shape[0]
    n_pad = N_CORES * NODES_PC
    W64 = feat_weights.astype(np.float64)
    wa = (W64 @ attn_weights.astype(np.float64)).reshape(D)
    wa32 = wa.astype(np.float32)

    neigh_p = np.zeros((n_pad, D * K), BF)
    nv = neigh_vecs.reshape(n, K, D)
    CH = 8192
    for i0 in range(0, n, CH):
        i1 = min(i0 + CH, n)
        blk = (nv[i0:i1] * wa32).transpose(0, 2, 1)  # [c, D, K] f32
        neigh_p[i0:i1] = np.ascontiguousarray(blk).reshape(i1 - i0, D * K).astype(BF)

    ss = np.zeros((n_pad, D), BF)
    ss[:n] = (self_vecs * wa32).astype(BF)
    w2 = (W64 / wa32.astype(np.float64)[:, None]).astype(BF)
    return neigh_p, ss, w2


def prep_in_maps(self_vecs, neigh_vecs, feat_weights, attn_weights, bias):
    neigh_p, ss, w2 = _prep(
        self_vecs, neigh_vecs, feat_weights, attn_weights, bias
    )
    mk = {
        "w2_bf": w2,
        "ident_bf": np.eye(P, dtype=np.float32).astype(BF),
        "ones_bf": np.ones((1, P), np.float32).astype(BF),
        "bias_bf": bias.reshape(1, D).astype(BF),
    }
    per_core = [
        {
            "neigh_bf": neigh_p[c * NODES_PC : (c + 1) * NODES_PC],
            "ss_bf": ss[c * NODES_PC : (c + 1) * NODES_PC],
        }
        for c in range(N_CORES)
    ]
    return mk, per_core


def kernel(self_vecs, neigh_vecs, feat_weights, attn_weights, bias, num_neighbors):
    self_vecs = np.asarray(self_vecs, dtype=np.float32)
    neigh_vecs = np.asarray(neigh_vecs, dtype=np.float32)
    feat_weights = np.asarray(feat_weights, dtype=np.float32)
    attn_weights = np.asarray(attn_weights, dtype=np.float32)
    bias = np.asarray(bias, dtype=np.float32)
    n = self_vecs.shape[0]

    mk, per_core = prep_in_maps(
        self_vecs, neigh_vecs, feat_weights, attn_weights, bias
    )

    if "nc" not in _cache:
        _cache["nc"] = _build()
    nc = _cache["nc"]

    in_maps = []
    for c in range(N_CORES):
        m = dict(per_core[c])
        m.update(mk)
        in_maps.append(m)

    import os

    trace = os.environ.get("KERNEL_TRACE") == "1"
    res = run_bass_kernel_spmd(nc, in_maps, list(range(N_CORES)), trace=trace)
    _cache["last_result"] = res
    out = np.concatenate([res.results[c]["out"] for c in range(N_CORES)], axis=0)
    return out[:n].astype(np.float32)
